# revision 1
# baseline (speedup 1.0000x reference)
"""LlamaAttention (B=2,S=2048,D=4096,H=32,KV=8) on 8 NeuronCores.

Sharding: tensor-parallel over heads. Core c owns Q heads 4c..4c+3 and KV
head c (GQA n_rep=4 means those Q heads all read exactly KV head c).
Per core: QKV projections (feature-major), RoPE, causal flash attention in
transposed-score layout (scores kept as S^T[k,q] so the softmax reduction
is a matmul-with-ones over the partition axis), then AllGather of the
per-core attention output (feature-major) and a column-sharded wo matmul.
Final output slices are concatenated host-side.

All on-chip matmul data is bf16 (fp32 accumulation in PSUM).
"""
import sys
import math

sys.path.insert(0, "/opt/trn_rl_repo")

import numpy as np
from ml_dtypes import bfloat16

B, S, D = 2, 2048, 4096
H, KVH, HD = 32, 8, 128
T = B * S                      # 4096 tokens
NC = 8                         # cores
HPC = H // NC                  # 4 q heads / core
QC = HPC * HD                  # 512 q-proj cols / core
TBS = 512                      # token block size
NTB = T // TBS                 # 8 token blocks
NDC = D // 128                 # 32 contraction chunks
NQB = S // TBS                 # 4 q blocks per sequence
ROPE_THETA = 10000.0

_CACHE = {}


def _build_nc():
    from concourse import bacc, tile, mybir

    f32 = mybir.dt.float32
    bf16 = mybir.dt.bfloat16
    ADD = mybir.AluOpType.add
    MULT = mybir.AluOpType.mult
    EXP = mybir.ActivationFunctionType.Exp
    COPY = mybir.ActivationFunctionType.Copy

    nc = bacc.Bacc("TRN2", target_bir_lowering=False, debug=False,
                   enable_asserts=True, num_devices=NC)

    hiddenT = nc.dram_tensor("hiddenT", [D, T], bf16, kind="ExternalInput").ap()
    wq_d = nc.dram_tensor("wq", [D, QC], bf16, kind="ExternalInput").ap()
    wk_d = nc.dram_tensor("wk", [D, HD], bf16, kind="ExternalInput").ap()
    wv_d = nc.dram_tensor("wv", [D, HD], bf16, kind="ExternalInput").ap()
    wo_d = nc.dram_tensor("wo", [D, QC], bf16, kind="ExternalInput").ap()
    cos_d = nc.dram_tensor("cosT", [HD, S], bf16, kind="ExternalInput").ap()
    sin_d = nc.dram_tensor("sinT", [HD, S], bf16, kind="ExternalInput").ap()
    # signed sin: row d holds -sin for d<64, +sin for d>=64 (rotate_half sign)
    ssin_d = nc.dram_tensor("ssinT", [HD, S], bf16, kind="ExternalInput").ap()
    maskb_d = nc.dram_tensor("maskb", [S, TBS], bf16, kind="ExternalInput").ap()
    ident_d = nc.dram_tensor("ident", [128, 128], bf16, kind="ExternalInput").ap()
    outT = nc.dram_tensor("outT", [QC, T], f32, kind="ExternalOutput").ap()

    with tile.TileContext(nc) as tc:
        with tc.tile_pool(name="persist", bufs=1) as pp, \
             tc.tile_pool(name="dram", bufs=1, space="DRAM") as dram:
            # ---- persistent SBUF tensors (live across phases) ----
            qt_sb = pp.tile([128, HPC * T], bf16, tag="qt")     # QT per head
            kt_sb = pp.tile([128, T], bf16, tag="kt")           # KT
            v_sb = pp.tile([128, (T // 128) * 128], bf16, tag="v")  # V token-major
            maskb_sb = pp.tile([128, 16 * TBS], bf16, tag="maskb")
            ident_sb = pp.tile([128, 128], bf16, tag="ident")
            ones_sb = pp.tile([128, 1], bf16, tag="ones")
            onesr_sb = pp.tile([1, 128], bf16, tag="onesr")

            nc.sync.dma_start(ident_sb[:], ident_d[:])
            nc.vector.memset(ones_sb[:], 1.0)
            nc.vector.memset(onesr_sb[:], 1.0)
            # mask diag band: tile (qb, j) = maskT rows [512qb+128j, +128),
            # cols local q of block qb
            for qb in range(NQB):
                for j in range(4):
                    nc.sync.dma_start(
                        maskb_sb[:, (4 * qb + j) * TBS:(4 * qb + j + 1) * TBS],
                        maskb_d[TBS * qb + 128 * j:TBS * qb + 128 * (j + 1), :])

            cc_in = [dram.tile([QC, S], bf16, tag=f"ccin{b}", name=f"ccin{b}") for b in range(B)]
            cc_out = [dram.tile([D, S], bf16, addr_space="Shared", tag=f"ccout{b}",
                                 name=f"ccout{b}") for b in range(B)]

            # ================= Phase 1: QKV + RoPE =================
            with tc.tile_pool(name="w1", bufs=1) as wp, \
                 tc.tile_pool(name="hp", bufs=4) as hp, \
                 tc.tile_pool(name="rope", bufs=3) as rp, \
                 tc.tile_pool(name="qkvps", bufs=6, space="PSUM") as qkv_ps, \
                 tc.tile_pool(name="trps", bufs=2, space="PSUM") as tr_ps:
                wq_sb = wp.tile([128, NDC * QC], bf16, tag="wq")
                wk_sb = wp.tile([128, NDC * HD], bf16, tag="wk")
                wv_sb = wp.tile([128, NDC * HD], bf16, tag="wv")
                cos_sb = wp.tile([128, S], bf16, tag="cos")
                ssin_sb = wp.tile([128, S], bf16, tag="ssin")
                for dc in range(NDC):
                    nc.sync.dma_start(wq_sb[:, QC * dc:QC * (dc + 1)],
                                      wq_d[128 * dc:128 * (dc + 1), :])
                    nc.sync.dma_start(wk_sb[:, HD * dc:HD * (dc + 1)],
                                      wk_d[128 * dc:128 * (dc + 1), :])
                    nc.sync.dma_start(wv_sb[:, HD * dc:HD * (dc + 1)],
                                      wv_d[128 * dc:128 * (dc + 1), :])
                nc.sync.dma_start(cos_sb[:], cos_d[:])
                nc.sync.dma_start(ssin_sb[:], ssin_d[:])

                for tb in range(NTB):
                    t0 = TBS * tb
                    ps_q = [qkv_ps.tile([128, TBS], f32, tag="qkv", name=f"psq{tb}_{i}") for i in range(HPC)]
                    ps_k = qkv_ps.tile([128, TBS], f32, tag="qkv")
                    ps_v = qkv_ps.tile([128, TBS], f32, tag="qkv")
                    for dc in range(NDC):
                        ht = hp.tile([128, TBS], bf16, tag="ht")
                        nc.sync.dma_start(ht[:], hiddenT[128 * dc:128 * (dc + 1), t0:t0 + TBS])
                        st, sp = dc == 0, dc == NDC - 1
                        for hl in range(HPC):
                            nc.tensor.matmul(ps_q[hl][:],
                                             wq_sb[:, QC * dc + HD * hl:QC * dc + HD * (hl + 1)],
                                             ht[:], start=st, stop=sp)
                        nc.tensor.matmul(ps_k[:], wk_sb[:, HD * dc:HD * (dc + 1)],
                                         ht[:], start=st, stop=sp)
                        nc.tensor.matmul(ps_v[:], wv_sb[:, HD * dc:HD * (dc + 1)],
                                         ht[:], start=st, stop=sp)

                    pos = TBS * (tb % NQB)   # position within sequence
                    cs = cos_sb[:, pos:pos + TBS]
                    ss = ssin_sb[:, pos:pos + TBS]
                    # RoPE for q heads and k
                    for idx in range(HPC + 1):
                        ps = ps_q[idx] if idx < HPC else ps_k
                        xf = rp.tile([128, TBS], f32, tag="xf")
                        nc.scalar.activation(xf[:], ps[:], COPY)
                        rot = rp.tile([128, TBS], f32, tag="rot")
                        nc.sync.dma_start(rot[0:64, :], xf[64:128, :])
                        nc.sync.dma_start(rot[64:128, :], xf[0:64, :])
                        t1 = rp.tile([128, TBS], f32, tag="t1")
                        nc.vector.tensor_tensor(t1[:], xf[:], cs, op=MULT)
                        t2 = rp.tile([128, TBS], f32, tag="t2")
                        nc.vector.tensor_tensor(t2[:], rot[:], ss, op=MULT)
                        if idx < HPC:
                            dst = qt_sb[:, T * idx + t0:T * idx + t0 + TBS]
                        else:
                            dst = kt_sb[:, t0:t0 + TBS]
                        nc.vector.tensor_tensor(dst, t1[:], t2[:], op=ADD)
                    # V: copy then transpose to token-major
                    vtmp = rp.tile([128, TBS], bf16, tag="vtmp")
                    nc.scalar.activation(vtmp[:], ps_v[:], COPY)
                    for j in range(TBS // 128):
                        tp = tr_ps.tile([128, 128], bf16, tag="tr")
                        with nc.allow_low_precision(reason="PE transpose, no accum"):
                            nc.tensor.transpose(tp[:], vtmp[:, 128 * j:128 * (j + 1)],
                                                ident_sb[:])
                        nc.vector.tensor_copy(
                            v_sb[:, t0 + 128 * j:t0 + 128 * (j + 1)], tp[:])

            # ================= Phase 2: attention =================
            with tc.tile_pool(name="sps", bufs=3, space="PSUM") as sps, \
                 tc.tile_pool(name="ops", bufs=2, space="PSUM") as ops, \
                 tc.tile_pool(name="dps", bufs=2, space="PSUM") as dps, \
                 tc.tile_pool(name="bps", bufs=1, space="PSUM") as bps, \
                 tc.tile_pool(name="ep", bufs=4) as ep, \
                 tc.tile_pool(name="np_", bufs=2) as np_:
                for b in range(B):
                    for hl in range(HPC):
                        qbase = T * hl + S * b
                        for qb in range(NQB):
                            q0 = TBS * qb
                            o_ps = ops.tile([128, TBS], f32, tag="o")
                            den = dps.tile([1, TBS], f32, tag="den")
                            nkc = 4 * (qb + 1)
                            for kc in range(nkc):
                                s_ps = sps.tile([128, TBS], f32, tag="s")
                                nc.tensor.matmul(
                                    s_ps[:],
                                    kt_sb[:, S * b + 128 * kc:S * b + 128 * (kc + 1)],
                                    qt_sb[:, qbase + q0:qbase + q0 + TBS],
                                    start=True, stop=True)
                                if 128 * (kc + 1) > TBS * qb:  # diagonal block
                                    j = kc - 4 * qb
                                    nc.vector.tensor_tensor(
                                        s_ps[:], s_ps[:],
                                        maskb_sb[:, (4 * qb + j) * TBS:(4 * qb + j + 1) * TBS],
                                        op=ADD)
                                ex = ep.tile([128, TBS], bf16, tag="ex")
                                nc.scalar.activation(ex[:], s_ps[:], EXP)
                                st, sp = kc == 0, kc == nkc - 1
                                nc.tensor.matmul(den[:], ones_sb[:, 0:1], ex[:],
                                                 start=st, stop=sp)
                                nc.tensor.matmul(
                                    o_ps[:],
                                    v_sb[:, S * b + 128 * kc:S * b + 128 * (kc + 1)],
                                    ex[:], start=st, stop=sp)
                            recip = np_.tile([1, TBS], bf16, tag="recip")
                            with nc.allow_low_precision(reason="bf16 softmax denom"):
                                nc.vector.reciprocal(recip[:], den[:])
                            b_ps = bps.tile([128, TBS], f32, tag="b")
                            nc.tensor.matmul(b_ps[:], onesr_sb[:], recip[:],
                                             start=True, stop=True)
                            bc = np_.tile([128, TBS], f32, tag="bc")
                            nc.scalar.activation(bc[:], b_ps[:], COPY)
                            at = np_.tile([128, TBS], bf16, tag="at")
                            nc.vector.tensor_tensor(at[:], o_ps[:], bc[:], op=MULT)
                            nc.sync.dma_start(
                                cc_in[b][128 * hl:128 * (hl + 1), q0:q0 + TBS], at[:])
                    # AllGather this batch's attention output (overlaps b=1 work)
                    nc.gpsimd.collective_compute(
                        "AllGather", mybir.AluOpType.bypass,
                        replica_groups=[list(range(NC))],
                        ins=[cc_in[b].opt()], outs=[cc_out[b].opt()])

            # ================= Phase 4: out projection =================
            with tc.tile_pool(name="w4", bufs=1) as wp4, \
                 tc.tile_pool(name="ap4", bufs=4) as ap4, \
                 tc.tile_pool(name="oc4", bufs=3) as oc4, \
                 tc.tile_pool(name="outps", bufs=4, space="PSUM") as out_ps:
                wo_sb = wp4.tile([128, NDC * QC], bf16, tag="wo")
                for dc in range(NDC):
                    nc.sync.dma_start(wo_sb[:, QC * dc:QC * (dc + 1)],
                                      wo_d[128 * dc:128 * (dc + 1), :])
                for tb in range(NTB):
                    b = tb // NQB
                    q0 = TBS * (tb % NQB)
                    ps_o = [out_ps.tile([128, TBS], f32, tag="po", name=f"pso{tb}_{i}") for i in range(4)]
                    for fc in range(NDC):
                        at = ap4.tile([128, TBS], bf16, tag="a4")
                        nc.sync.dma_start(at[:], cc_out[b][128 * fc:128 * (fc + 1),
                                                           q0:q0 + TBS])
                        st, sp = fc == 0, fc == NDC - 1
                        for nt in range(4):
                            nc.tensor.matmul(
                                ps_o[nt][:],
                                wo_sb[:, QC * fc + 128 * nt:QC * fc + 128 * (nt + 1)],
                                at[:], start=st, stop=sp)
                    for nt in range(4):
                        oc = oc4.tile([128, TBS], f32, tag="oc")
                        nc.scalar.activation(oc[:], ps_o[nt][:], COPY)
                        nc.sync.dma_start(
                            outT[128 * nt:128 * (nt + 1), TBS * tb:TBS * (tb + 1)],
                            oc[:])

    nc.compile()
    return nc


def _stage_inputs(hidden_states, wq, wk, wv, wo, attention_mask):
    hid = np.asarray(hidden_states, dtype=np.float32).reshape(T, D)
    hiddenT = np.ascontiguousarray(hid.T).astype(bfloat16)

    sc = 1.0 / math.sqrt(HD)
    mask = np.asarray(attention_mask, dtype=np.float32).reshape(S, S)
    # diag band, transposed: rows k in [512qb,512qb+512), cols q local
    maskb = np.concatenate(
        [np.ascontiguousarray(mask[TBS * qb:TBS * (qb + 1),
                                   TBS * qb:TBS * (qb + 1)].T)
         for qb in range(NQB)], axis=0).astype(bfloat16)

    inv_freq = 1.0 / (ROPE_THETA ** (np.arange(0, HD, 2, dtype=np.float32) / HD))
    t = np.arange(S, dtype=np.float32)
    freqs = np.outer(t, inv_freq)
    emb = np.concatenate([freqs, freqs], axis=-1)        # [S, HD]
    cosT = np.ascontiguousarray(np.cos(emb).T).astype(bfloat16)   # [HD, S]
    sinT = np.ascontiguousarray(np.sin(emb).T)
    ssinT = sinT.copy()
    ssinT[:HD // 2] *= -1.0       # rotate_half sign: -sin for d<64
    sinT = sinT.astype(bfloat16)
    ssinT = ssinT.astype(bfloat16)
    ident = np.eye(128, dtype=np.float32).astype(bfloat16)

    wq = np.asarray(wq, dtype=np.float32)
    wk = np.asarray(wk, dtype=np.float32)
    wv = np.asarray(wv, dtype=np.float32)
    wo = np.asarray(wo, dtype=np.float32)

    in_maps = []
    for c in range(NC):
        in_maps.append({
            "hiddenT": hiddenT,
            "wq": np.ascontiguousarray(wq[:, QC * c:QC * (c + 1)] * sc).astype(bfloat16),
            "wk": np.ascontiguousarray(wk[:, HD * c:HD * (c + 1)]).astype(bfloat16),
            "wv": np.ascontiguousarray(wv[:, HD * c:HD * (c + 1)]).astype(bfloat16),
            "wo": np.ascontiguousarray(wo[:, QC * c:QC * (c + 1)]).astype(bfloat16),
            "cosT": cosT, "sinT": sinT, "ssinT": ssinT,
            "maskb": maskb, "ident": ident,
        })
    return in_maps


def kernel(hidden_states, wq, wk, wv, wo, attention_mask, _want_trace=False):
    from concourse import bass_utils

    if "nc" not in _CACHE:
        _CACHE["nc"] = _build_nc()
    nc = _CACHE["nc"]

    in_maps = _stage_inputs(hidden_states, wq, wk, wv, wo, attention_mask)
    res = bass_utils.run_bass_kernel_spmd(
        nc, in_maps, core_ids=list(range(NC)), trace=_want_trace)
    _CACHE["last_result"] = res

    outT_full = np.concatenate([res.results[c]["outT"] for c in range(NC)], axis=0)
    out = np.ascontiguousarray(outT_full.T).reshape(B, S, D).astype(np.float32)
    return out



# revision 2
# speedup vs baseline: 1.4378x; 1.4378x over previous
"""LlamaAttention (B=2,S=2048,D=4096,H=32,KV=8) on 8 NeuronCores.

Tensor-parallel over heads, fused single-pass pipeline, no collectives.

Core c owns Q heads 4c..4c+3 and KV head c (GQA n_rep=4 -> those Q heads
read exactly KV head c). Per 512-token block tb (8 blocks = 2 batches x 4
q-blocks), each core runs:
  1. QKV projections (order: k, v, q0..q3; weights stationary, hidden
     moving, fp32 PSUM accumulation over 32 contraction chunks),
  2. RoPE on k and q heads (scalar copy to bf16 + DMA half-swap + DVE
     mul/mul/add), V transpose to token-major via PE transposes,
  3. causal flash attention in transposed-score layout (scores kept as
     S^T[k,q]); the causal diagonal is trimmed by slicing the matmul free
     dim, and the within-tile triangle is applied as a 0/1 bf16 multiply
     after exp,
  4. partial out-projection: row-shard of wo (rows 512c..512c+512)
     contracted against the core's 4 attention heads -> partial output
     [4096, 512] for this token block, written to DRAM as bf16.

The 8 per-core partial outputs are summed on the host (the row-sharded
wo reduction), replacing the AllGather + column-sharded wo of the
previous version: zero collective time and no DRAM round trip for the
gathered activations. All on-chip matmul data is bf16 (fp32 PSUM).
"""
import sys
import math

sys.path.insert(0, "/opt/trn_rl_repo")

import numpy as np
from ml_dtypes import bfloat16

B, S, D = 2, 2048, 4096
H, KVH, HD = 32, 8, 128
T = B * S                      # 4096 tokens
NC = 8                         # cores
HPC = H // NC                  # 4 q heads / core
QC = HPC * HD                  # 512 q-proj cols / core
TBS = 512                      # token block size
NTB = T // TBS                 # 8 token blocks
NDC = D // 128                 # 32 contraction chunks
NQB = S // TBS                 # 4 q blocks per sequence
DCG = 4                        # contraction chunks per hidden DMA tile
ROPE_THETA = 10000.0

_CACHE = {}


def _build_nc():
    from concourse import bacc, tile, mybir

    f32 = mybir.dt.float32
    bf16 = mybir.dt.bfloat16
    MULT = mybir.AluOpType.mult
    ADD = mybir.AluOpType.add
    EXP = mybir.ActivationFunctionType.Exp
    COPY = mybir.ActivationFunctionType.Copy

    nc = bacc.Bacc("TRN2", target_bir_lowering=False, debug=False,
                   enable_asserts=True, num_devices=NC)

    hid_d = nc.dram_tensor("hiddenT", [128, NTB * NDC * TBS], bf16,
                           kind="ExternalInput").ap()
    wq_d = nc.dram_tensor("wq", [128, NDC * QC], bf16, kind="ExternalInput").ap()
    wk_d = nc.dram_tensor("wk", [128, NDC * HD], bf16, kind="ExternalInput").ap()
    wv_d = nc.dram_tensor("wv", [128, NDC * HD], bf16, kind="ExternalInput").ap()
    wo_d = nc.dram_tensor("wo", [128, HPC * D], bf16, kind="ExternalInput").ap()
    cos_d = nc.dram_tensor("cosT", [HD, S], bf16, kind="ExternalInput").ap()
    # signed sin: row d holds -sin for d<64, +sin for d>=64 (rotate_half sign)
    ssin_d = nc.dram_tensor("ssinT", [HD, S], bf16, kind="ExternalInput").ap()
    tri_d = nc.dram_tensor("tri", [128, 128], bf16, kind="ExternalInput").ap()
    ident_d = nc.dram_tensor("ident", [128, 128], bf16, kind="ExternalInput").ap()
    outT = nc.dram_tensor("outT", [D, T], bf16, kind="ExternalOutput").ap()

    with tile.TileContext(nc) as tc:
        with tc.tile_pool(name="persist", bufs=1) as pp, \
             tc.tile_pool(name="hp", bufs=12) as hp, \
             tc.tile_pool(name="rp", bufs=8) as rp, \
             tc.tile_pool(name="ep", bufs=6) as ep, \
             tc.tile_pool(name="np_", bufs=2) as np_, \
             tc.tile_pool(name="oc", bufs=8) as oc, \
             tc.tile_pool(name="ps_qkv", bufs=2, space="PSUM") as ps_qkv, \
             tc.tile_pool(name="ps_s", bufs=2, space="PSUM") as ps_s, \
             tc.tile_pool(name="ps_o", bufs=1, space="PSUM") as ps_o, \
             tc.tile_pool(name="ps_den", bufs=1, space="PSUM") as ps_den, \
             tc.tile_pool(name="ps_wo", bufs=2, space="PSUM") as ps_wo:

            # ---- persistent SBUF tensors ----
            wq_sb = pp.tile([128, NDC * QC], bf16, tag="wq")
            wk_sb = pp.tile([128, NDC * HD], bf16, tag="wk")
            wv_sb = pp.tile([128, NDC * HD], bf16, tag="wv")
            wo_sb = pp.tile([128, HPC * D], bf16, tag="wo")
            cos_sb = pp.tile([128, S], bf16, tag="cos")
            ssin_sb = pp.tile([128, S], bf16, tag="ssin")
            tri_sb = pp.tile([128, 128], bf16, tag="tri")
            ident_sb = pp.tile([128, 128], bf16, tag="ident")
            ones_sb = pp.tile([128, 1], bf16, tag="ones")
            onesr_sb = pp.tile([1, 128], bf16, tag="onesr")
            kt_sb = pp.tile([128, S], bf16, tag="kt")      # K^T, current batch
            v_sb = pp.tile([128, S], bf16, tag="v")        # V token-major, cur batch
            qt_sb = pp.tile([128, HPC * TBS], bf16, tag="qt")   # Q^T, current tb
            at_sb = pp.tile([128, HPC * TBS], bf16, tag="at")   # attn out, cur tb

            # ---- prologue DMAs (ordered by first use) ----
            nc.vector.memset(ones_sb[:], 1.0)
            nc.vector.memset(onesr_sb[:], 1.0)
            for g in range(2):
                nc.sync.dma_start(wk_sb[:, 2048 * g:2048 * (g + 1)],
                                  wk_d[:, 2048 * g:2048 * (g + 1)])
                nc.sync.dma_start(wv_sb[:, 2048 * g:2048 * (g + 1)],
                                  wv_d[:, 2048 * g:2048 * (g + 1)])
            nc.sync.dma_start(cos_sb[:], cos_d[:])
            nc.sync.dma_start(ssin_sb[:], ssin_d[:])
            nc.sync.dma_start(tri_sb[:], tri_d[:])
            nc.sync.dma_start(ident_sb[:], ident_d[:])

            ht_tiles = {}

            def prefetch_tb(tb):
                tiles = []
                for g in range(NDC // DCG):
                    ht = hp.tile([128, DCG * TBS], bf16, tag="ht",
                                 name=f"ht{tb}_{g}")
                    nc.sync.dma_start(
                        ht[:],
                        hid_d[:, (tb * NDC + g * DCG) * TBS:
                              (tb * NDC + (g + 1) * DCG) * TBS])
                    tiles.append(ht)
                ht_tiles[tb] = tiles

            prefetch_tb(0)
            # interleave wq chunks with wo chunks (wq needed first)
            for g in range(8):
                nc.sync.dma_start(wq_sb[:, 2048 * g:2048 * (g + 1)],
                                  wq_d[:, 2048 * g:2048 * (g + 1)])
            for g in range(8):
                nc.sync.dma_start(wo_sb[:, 2048 * g:2048 * (g + 1)],
                                  wo_d[:, 2048 * g:2048 * (g + 1)])

            def rope(ps, dst, pos0):
                """dst (bf16) = rope(ps); pos0 = seq position of column 0."""
                cs = cos_sb[:, pos0:pos0 + TBS]
                ss = ssin_sb[:, pos0:pos0 + TBS]
                xf = rp.tile([128, TBS], bf16, tag="rp")
                nc.scalar.activation(xf[:], ps[:], COPY)
                rot = rp.tile([128, TBS], bf16, tag="rp")
                nc.sync.dma_start(rot[0:64, :], xf[64:128, :])
                nc.sync.dma_start(rot[64:128, :], xf[0:64, :])
                t1 = rp.tile([128, TBS], bf16, tag="rp")
                nc.vector.tensor_tensor(t1[:], xf[:], cs, op=MULT)
                t2 = rp.tile([128, TBS], bf16, tag="rp")
                nc.vector.tensor_tensor(t2[:], rot[:], ss, op=MULT)
                nc.vector.tensor_tensor(dst, t1[:], t2[:], op=ADD)

            for tb in range(NTB):
                b, qb = tb // NQB, tb % NQB
                pos0 = qb * TBS
                if tb + 1 < NTB:
                    prefetch_tb(tb + 1)

                # ---- QKV projections: k, v, q0, q1 ... then V transpose, q2, q3
                def qkv_mm(lhs_fn, name):
                    ps = ps_qkv.tile([128, TBS], f32, tag="qkv", name=name)
                    for dc in range(NDC):
                        ht = ht_tiles[tb][dc // DCG]
                        rhs = ht[:, (dc % DCG) * TBS:(dc % DCG + 1) * TBS]
                        nc.tensor.matmul(ps[:], lhs_fn(dc), rhs,
                                         start=dc == 0, stop=dc == NDC - 1)
                    return ps

                ps_k = qkv_mm(lambda dc: wk_sb[:, HD * dc:HD * (dc + 1)], f"psk{tb}")
                rope(ps_k, kt_sb[:, pos0:pos0 + TBS], pos0)

                ps_v = qkv_mm(lambda dc: wv_sb[:, HD * dc:HD * (dc + 1)], f"psv{tb}")
                vtmp = rp.tile([128, TBS], bf16, tag="vtmp")
                nc.scalar.activation(vtmp[:], ps_v[:], COPY)

                ps_q0 = qkv_mm(
                    lambda dc: wq_sb[:, QC * dc:QC * dc + HD], f"psq{tb}_0")
                rope(ps_q0, qt_sb[:, 0:TBS], pos0)

                # V transpose to token-major (PE transposes, psum via "s" tag)
                for j in range(TBS // 128):
                    tp = ps_s.tile([128, 128], bf16, tag="s", name=f"tr{tb}_{j}")
                    with nc.allow_low_precision(reason="PE transpose, no accum"):
                        nc.tensor.transpose(tp[:], vtmp[:, 128 * j:128 * (j + 1)],
                                            ident_sb[:])
                    nc.vector.tensor_copy(
                        v_sb[:, pos0 + 128 * j:pos0 + 128 * (j + 1)], tp[:])

                for hl in range(1, HPC):
                    ps_q = qkv_mm(
                        lambda dc: wq_sb[:, QC * dc + HD * hl:QC * dc + HD * (hl + 1)],
                        f"psq{tb}_{hl}")
                    rope(ps_q, qt_sb[:, TBS * hl:TBS * (hl + 1)], pos0)

                # ---- attention for (b, qb), all 4 local heads ----
                nkc = 4 * (qb + 1)
                for hl in range(HPC):
                    o_ps = ps_o.tile([128, TBS], f32, tag="o")
                    den = ps_den.tile([1, TBS], f32, tag="den")
                    for kc in range(nkc):
                        j = kc - 4 * qb
                        off = 128 * j if j > 0 else 0
                        s_ps = ps_s.tile([128, TBS], f32, tag="s")
                        nc.tensor.matmul(
                            s_ps[:, off:TBS],
                            kt_sb[:, 128 * kc:128 * (kc + 1)],
                            qt_sb[:, TBS * hl + off:TBS * (hl + 1)],
                            start=True, stop=True)
                        ex = ep.tile([128, TBS], bf16, tag="ex")
                        nc.scalar.activation(ex[:, off:TBS], s_ps[:, off:TBS], EXP)
                        if j >= 0:  # apply within-tile causal triangle
                            nc.vector.tensor_tensor(
                                ex[:, off:off + 128], ex[:, off:off + 128],
                                tri_sb[:], op=MULT)
                        st, sp = kc == 0, kc == nkc - 1
                        nc.tensor.matmul(den[:, off:TBS], ones_sb[:, 0:1],
                                         ex[:, off:TBS], start=st, stop=sp)
                        nc.tensor.matmul(
                            o_ps[:, off:TBS],
                            v_sb[:, 128 * kc:128 * (kc + 1)],
                            ex[:, off:TBS], start=st, stop=sp)
                    recip = np_.tile([1, TBS], bf16, tag="recip")
                    with nc.allow_low_precision(reason="bf16 softmax denom"):
                        nc.vector.reciprocal(recip[:], den[:])
                    b_ps = ps_s.tile([128, TBS], f32, tag="s", name=f"bps{tb}_{hl}")
                    nc.tensor.matmul(b_ps[:], onesr_sb[:], recip[:],
                                     start=True, stop=True)
                    bc = np_.tile([128, TBS], bf16, tag="bc")
                    nc.scalar.activation(bc[:], b_ps[:], COPY)
                    nc.vector.tensor_tensor(at_sb[:, TBS * hl:TBS * (hl + 1)],
                                            o_ps[:], bc[:], op=MULT)

                # ---- partial out-projection for this token block ----
                for nt in range(D // 128):
                    wo_ps = ps_wo.tile([128, TBS], f32, tag="wo")
                    for hl in range(HPC):
                        nc.tensor.matmul(
                            wo_ps[:],
                            wo_sb[:, D * hl + 128 * nt:D * hl + 128 * (nt + 1)],
                            at_sb[:, TBS * hl:TBS * (hl + 1)],
                            start=hl == 0, stop=hl == HPC - 1)
                    oc_t = oc.tile([128, TBS], bf16, tag="oc")
                    if nt % 2 == 0:
                        nc.scalar.activation(oc_t[:], wo_ps[:], COPY)
                    else:
                        nc.vector.tensor_copy(oc_t[:], wo_ps[:])
                    nc.sync.dma_start(
                        outT[128 * nt:128 * (nt + 1), TBS * tb:TBS * (tb + 1)],
                        oc_t[:])

    nc.compile()
    return nc


def _stage_inputs(hidden_states, wq, wk, wv, wo, attention_mask):
    hid = np.asarray(hidden_states, dtype=np.float32).reshape(T, D)
    # [128, (tb, dc, t_local)] : column (tb*32+dc)*512+tl = hid[tb*512+tl, dc*128+p]
    hiddenT = np.ascontiguousarray(
        hid.reshape(NTB, TBS, NDC, 128).transpose(3, 0, 2, 1).reshape(128, -1)
    ).astype(bfloat16)

    sc = 1.0 / math.sqrt(HD)
    inv_freq = 1.0 / (ROPE_THETA ** (np.arange(0, HD, 2, dtype=np.float32) / HD))
    t = np.arange(S, dtype=np.float32)
    freqs = np.outer(t, inv_freq)
    emb = np.concatenate([freqs, freqs], axis=-1)          # [S, HD]
    cosT = np.ascontiguousarray(np.cos(emb).T).astype(bfloat16)   # [HD, S]
    ssinT = np.ascontiguousarray(np.sin(emb).T)
    ssinT[:HD // 2] *= -1.0        # rotate_half sign: -sin for d<64
    ssinT = ssinT.astype(bfloat16)

    # 0/1 within-tile causal triangle: tri[k,q] = 1 iff key k <= query q
    mask = np.asarray(attention_mask, dtype=np.float32).reshape(S, S)
    tri = (mask[0:128, 0:128].T > -0.5).astype(np.float32).astype(bfloat16)
    ident = np.eye(128, dtype=np.float32).astype(bfloat16)

    wq = np.asarray(wq, dtype=np.float32)
    wk = np.asarray(wk, dtype=np.float32)
    wv = np.asarray(wv, dtype=np.float32)
    wo = np.asarray(wo, dtype=np.float32)

    in_maps = []
    for c in range(NC):
        wq_c = (wq[:, QC * c:QC * (c + 1)] * sc)
        wq_c = np.ascontiguousarray(
            wq_c.reshape(NDC, 128, QC).transpose(1, 0, 2).reshape(128, -1)
        ).astype(bfloat16)
        wk_c = np.ascontiguousarray(
            wk[:, HD * c:HD * (c + 1)].reshape(NDC, 128, HD)
            .transpose(1, 0, 2).reshape(128, -1)).astype(bfloat16)
        wv_c = np.ascontiguousarray(
            wv[:, HD * c:HD * (c + 1)].reshape(NDC, 128, HD)
            .transpose(1, 0, 2).reshape(128, -1)).astype(bfloat16)
        wo_c = np.ascontiguousarray(
            wo[QC * c:QC * (c + 1), :].reshape(HPC, 128, D)
            .transpose(1, 0, 2).reshape(128, -1)).astype(bfloat16)
        in_maps.append({
            "hiddenT": hiddenT,
            "wq": wq_c, "wk": wk_c, "wv": wv_c, "wo": wo_c,
            "cosT": cosT, "ssinT": ssinT, "tri": tri, "ident": ident,
        })
    return in_maps


def kernel(hidden_states, wq, wk, wv, wo, attention_mask, _want_trace=False):
    from concourse import bass_utils

    if "nc" not in _CACHE:
        _CACHE["nc"] = _build_nc()
    nc = _CACHE["nc"]

    in_maps = _stage_inputs(hidden_states, wq, wk, wv, wo, attention_mask)
    res = bass_utils.run_bass_kernel_spmd(
        nc, in_maps, core_ids=list(range(NC)), trace=_want_trace)
    _CACHE["last_result"] = res

    # host-side reduction of the row-sharded wo partials
    acc = np.zeros((D, T), dtype=np.float32)
    for c in range(NC):
        acc += res.results[c]["outT"].astype(np.float32)
    out = np.ascontiguousarray(acc.T).reshape(B, S, D)
    return out


# revision 4
# speedup vs baseline: 1.7656x; 1.2280x over previous
"""LlamaAttention (B=2,S=2048,D=4096,H=32,KV=8) on 8 NeuronCores.

Tensor-parallel over heads, fused single-pass pipeline, no collectives.

Core c owns Q heads 4c..4c+3 and KV head c (GQA n_rep=4 -> those Q heads
read exactly KV head c). Per 512-token block tb (8 blocks = 2 batches x 4
q-blocks), each core runs:
  1. QKV projections (order: k, v, q0..q3; weights stationary, hidden
     moving, fp32 PSUM accumulation over 32 contraction chunks),
  2. RoPE on k and q heads (scalar copy to bf16 + DMA half-swap + DVE
     mul/mul/add), V transpose to token-major via PE transposes,
  3. causal flash attention in transposed-score layout (scores kept as
     S^T[k,q]); the causal diagonal is trimmed by slicing the matmul free
     dim, and the within-tile triangle is applied as a 0/1 bf16 multiply
     after exp,
  4. partial out-projection: row-shard of wo (rows 512c..512c+512)
     contracted against the core's 4 attention heads -> partial output
     [4096, 512] for this token block, written to DRAM as bf16.

The 8 per-core partial outputs are summed on the host (the row-sharded
wo reduction), replacing the AllGather + column-sharded wo of the
previous version: zero collective time and no DRAM round trip for the
gathered activations. All on-chip matmul data is bf16 (fp32 PSUM).
"""
import sys
import math

sys.path.insert(0, "/opt/trn_rl_repo")

import numpy as np
from ml_dtypes import bfloat16

B, S, D = 2, 2048, 4096
H, KVH, HD = 32, 8, 128
T = B * S                      # 4096 tokens
NC = 8                         # cores
HPC = H // NC                  # 4 q heads / core
QC = HPC * HD                  # 512 q-proj cols / core
TBS = 512                      # token block size
NTB = T // TBS                 # 8 token blocks
NDC = D // 128                 # 32 contraction chunks
NQB = S // TBS                 # 4 q blocks per sequence
DCG = 4                        # contraction chunks per hidden DMA tile
ROPE_THETA = 10000.0

_CACHE = {}


def _build_nc():
    from concourse import bacc, tile, mybir

    f32 = mybir.dt.float32
    bf16 = mybir.dt.bfloat16
    MULT = mybir.AluOpType.mult
    ADD = mybir.AluOpType.add
    EXP = mybir.ActivationFunctionType.Exp
    COPY = mybir.ActivationFunctionType.Copy

    nc = bacc.Bacc("TRN2", target_bir_lowering=False, debug=False,
                   enable_asserts=True, num_devices=NC)

    hid_d = nc.dram_tensor("hiddenT", [128, NTB * NDC * TBS], bf16,
                           kind="ExternalInput").ap()
    wq_d = nc.dram_tensor("wq", [128, NDC * QC], bf16, kind="ExternalInput").ap()
    wk_d = nc.dram_tensor("wk", [128, NDC * HD], bf16, kind="ExternalInput").ap()
    wv_d = nc.dram_tensor("wv", [128, NDC * HD], bf16, kind="ExternalInput").ap()
    wo_d = nc.dram_tensor("wo", [128, HPC * D], bf16, kind="ExternalInput").ap()
    cos_d = nc.dram_tensor("cosT", [HD, S], bf16, kind="ExternalInput").ap()
    # signed sin: row d holds -sin for d<64, +sin for d>=64 (rotate_half sign)
    ssin_d = nc.dram_tensor("ssinT", [HD, S], bf16, kind="ExternalInput").ap()
    tri_d = nc.dram_tensor("tri", [128, 128], bf16, kind="ExternalInput").ap()
    ident_d = nc.dram_tensor("ident", [128, 128], bf16, kind="ExternalInput").ap()
    outT = nc.dram_tensor("outT", [D, T], bf16, kind="ExternalOutput").ap()

    with tile.TileContext(nc) as tc:
        with tc.tile_pool(name="persist", bufs=1) as pp, \
             tc.tile_pool(name="hp", bufs=12) as hp, \
             tc.tile_pool(name="rp", bufs=8) as rp, \
             tc.tile_pool(name="ep", bufs=6) as ep, \
             tc.tile_pool(name="np_", bufs=2) as np_, \
             tc.tile_pool(name="oc", bufs=8) as oc, \
             tc.tile_pool(name="ps_qkv", bufs=2, space="PSUM") as ps_qkv, \
             tc.tile_pool(name="ps_s", bufs=2, space="PSUM") as ps_s, \
             tc.tile_pool(name="ps_o", bufs=1, space="PSUM") as ps_o, \
             tc.tile_pool(name="ps_den", bufs=1, space="PSUM") as ps_den, \
             tc.tile_pool(name="ps_wo", bufs=2, space="PSUM") as ps_wo:

            # ---- persistent SBUF tensors ----
            wq_sb = pp.tile([128, NDC * QC], bf16, tag="wq")
            # wk split in two tiles so the first 16 matmuls start after 1MB
            wk_sb = [pp.tile([128, NDC * HD // 2], bf16, tag=f"wk{h}",
                             name=f"wk_sb{h}") for h in range(2)]
            wv_sb = pp.tile([128, NDC * HD], bf16, tag="wv")
            wo_sb = pp.tile([128, HPC * D], bf16, tag="wo")
            cos_sb = pp.tile([128, S], bf16, tag="cos")
            ssin_sb = pp.tile([128, S], bf16, tag="ssin")
            tri_sb = pp.tile([128, 128], bf16, tag="tri")
            ident_sb = pp.tile([128, 128], bf16, tag="ident")
            ones_sb = pp.tile([128, 128], bf16, tag="ones")
            kt_sb = pp.tile([128, S], bf16, tag="kt")      # K^T, current batch
            v_sb = pp.tile([128, S], bf16, tag="v")        # V token-major, cur batch
            qt_sb = pp.tile([128, HPC * TBS], bf16, tag="qt")   # Q^T, current tb
            at_sb = pp.tile([128, HPC * TBS], bf16, tag="at")   # attn out, cur tb

            nc.vector.memset(ones_sb[:], 1.0)

            ht_tiles = {}

            def prefetch_tb(tb):
                tiles = []
                for g in range(NDC // DCG):
                    ht = hp.tile([128, DCG * TBS], bf16, tag="ht",
                                 name=f"ht{tb}_{g}")
                    nc.sync.dma_start(
                        ht[:],
                        hid_d[:, (tb * NDC + g * DCG) * TBS:
                              (tb * NDC + (g + 1) * DCG) * TBS])
                    tiles.append(ht)
                ht_tiles[tb] = tiles

            # ---- prologue DMAs (ordered by first use) ----
            nc.sync.dma_start(wk_sb[0][:], wk_d[:, 0:2048])
            prefetch_tb(0)
            nc.sync.dma_start(wk_sb[1][:], wk_d[:, 2048:4096])
            for g in range(2):
                nc.sync.dma_start(wv_sb[:, 2048 * g:2048 * (g + 1)],
                                  wv_d[:, 2048 * g:2048 * (g + 1)])
            nc.sync.dma_start(cos_sb[:], cos_d[:])
            nc.sync.dma_start(ssin_sb[:], ssin_d[:])
            nc.sync.dma_start(tri_sb[:], tri_d[:])
            nc.sync.dma_start(ident_sb[:], ident_d[:])
            for g in range(8):
                nc.sync.dma_start(wq_sb[:, 2048 * g:2048 * (g + 1)],
                                  wq_d[:, 2048 * g:2048 * (g + 1)])
            for g in range(8):
                nc.sync.dma_start(wo_sb[:, 2048 * g:2048 * (g + 1)],
                                  wo_d[:, 2048 * g:2048 * (g + 1)])

            def rope(ps, dst, pos0):
                """dst (bf16) = rope(ps); pos0 = seq position of column 0."""
                cs = cos_sb[:, pos0:pos0 + TBS]
                ss = ssin_sb[:, pos0:pos0 + TBS]
                xf = rp.tile([128, TBS], bf16, tag="rp")
                nc.scalar.activation(xf[:], ps[:], COPY)
                rot = rp.tile([128, TBS], bf16, tag="rp")
                nc.sync.dma_start(rot[0:64, :], xf[64:128, :])
                nc.sync.dma_start(rot[64:128, :], xf[0:64, :])
                t1 = rp.tile([128, TBS], bf16, tag="rp")
                nc.vector.tensor_tensor(t1[:], xf[:], cs, op=MULT)
                t2 = rp.tile([128, TBS], bf16, tag="rp")
                nc.vector.tensor_tensor(t2[:], rot[:], ss, op=MULT)
                nc.vector.tensor_tensor(dst, t1[:], t2[:], op=ADD)

            for tb in range(NTB):
                b, qb = tb // NQB, tb % NQB
                pos0 = qb * TBS
                if tb + 1 < NTB:
                    prefetch_tb(tb + 1)

                # ---- QKV projections: k, v, q0, q1 ... then V transpose, q2, q3
                def qkv_mm(lhs_fn, name):
                    ps = ps_qkv.tile([128, TBS], f32, tag="qkv", name=name)
                    for dc in range(NDC):
                        ht = ht_tiles[tb][dc // DCG]
                        rhs = ht[:, (dc % DCG) * TBS:(dc % DCG + 1) * TBS]
                        nc.tensor.matmul(ps[:], lhs_fn(dc), rhs,
                                         start=dc == 0, stop=dc == NDC - 1)
                    return ps

                def attn(hl):
                    """Attention for local head hl of (b, qb): transposed-score
                    flash pass; den matmul broadcasts the softmax denominator
                    across all 128 partitions (ones[128,128] lhsT) so the
                    reciprocal runs full-width off the tensor critical path."""
                    nkc = 4 * (qb + 1)
                    o_ps = ps_o.tile([128, TBS], f32, tag="o")
                    den = ps_den.tile([128, TBS], f32, tag="den")
                    for kc in range(nkc):
                        j = kc - 4 * qb
                        off = 128 * j if j > 0 else 0
                        s_ps = ps_s.tile([128, TBS], f32, tag="s")
                        nc.tensor.matmul(
                            s_ps[:, off:TBS],
                            kt_sb[:, 128 * kc:128 * (kc + 1)],
                            qt_sb[:, TBS * hl + off:TBS * (hl + 1)],
                            start=True, stop=True)
                        ex = ep.tile([128, TBS], bf16, tag="ex")
                        nc.scalar.activation(ex[:, off:TBS], s_ps[:, off:TBS], EXP)
                        if j >= 0:  # apply within-tile causal triangle
                            nc.vector.tensor_tensor(
                                ex[:, off:off + 128], ex[:, off:off + 128],
                                tri_sb[:], op=MULT)
                        st, sp = kc == 0, kc == nkc - 1
                        nc.tensor.matmul(den[:, off:TBS], ones_sb[:],
                                         ex[:, off:TBS], start=st, stop=sp)
                        nc.tensor.matmul(
                            o_ps[:, off:TBS],
                            v_sb[:, 128 * kc:128 * (kc + 1)],
                            ex[:, off:TBS], start=st, stop=sp)
                    recip = np_.tile([128, TBS], f32, tag="recip")
                    nc.vector.reciprocal_approx_fast(recip[:], den[:])
                    nc.vector.tensor_tensor(at_sb[:, TBS * hl:TBS * (hl + 1)],
                                            o_ps[:], recip[:], op=MULT)

                ps_k = qkv_mm(
                    lambda dc: wk_sb[dc // 16][:, HD * (dc % 16):HD * (dc % 16 + 1)],
                    f"psk{tb}")
                rope(ps_k, kt_sb[:, pos0:pos0 + TBS], pos0)

                ps_v = qkv_mm(lambda dc: wv_sb[:, HD * dc:HD * (dc + 1)], f"psv{tb}")
                vtmp = rp.tile([128, TBS], bf16, tag="vtmp")
                nc.scalar.activation(vtmp[:], ps_v[:], COPY)

                ps_q0 = qkv_mm(
                    lambda dc: wq_sb[:, QC * dc:QC * dc + HD], f"psq{tb}_0")
                rope(ps_q0, qt_sb[:, 0:TBS], pos0)

                # V transpose to token-major (PE transposes, psum via "s" tag)
                for j in range(TBS // 128):
                    tp = ps_s.tile([128, 128], bf16, tag="s", name=f"tr{tb}_{j}")
                    with nc.allow_low_precision(reason="PE transpose, no accum"):
                        nc.tensor.transpose(tp[:], vtmp[:, 128 * j:128 * (j + 1)],
                                            ident_sb[:])
                    nc.vector.tensor_copy(
                        v_sb[:, pos0 + 128 * j:pos0 + 128 * (j + 1)], tp[:])

                # interleave: q-head hl+1 projection, then attention of head hl
                # (exp/den/AV of head hl hide under head hl+1's matmul stream)
                for hl in range(HPC):
                    if hl + 1 < HPC:
                        ps_q = qkv_mm(
                            lambda dc, h=hl + 1:
                            wq_sb[:, QC * dc + HD * h:QC * dc + HD * (h + 1)],
                            f"psq{tb}_{hl + 1}")
                    attn(hl)
                    if hl + 1 < HPC:
                        rope(ps_q, qt_sb[:, TBS * (hl + 1):TBS * (hl + 2)], pos0)

                # ---- partial out-projection for this token block ----
                for nt in range(D // 128):
                    wo_ps = ps_wo.tile([128, TBS], f32, tag="wo")
                    for hl in range(HPC):
                        nc.tensor.matmul(
                            wo_ps[:],
                            wo_sb[:, D * hl + 128 * nt:D * hl + 128 * (nt + 1)],
                            at_sb[:, TBS * hl:TBS * (hl + 1)],
                            start=hl == 0, stop=hl == HPC - 1)
                    oc_t = oc.tile([128, TBS], bf16, tag="oc")
                    if nt % 2 == 0:
                        nc.scalar.activation(oc_t[:], wo_ps[:], COPY)
                    else:
                        nc.vector.tensor_copy(oc_t[:], wo_ps[:])
                    nc.sync.dma_start(
                        outT[128 * nt:128 * (nt + 1), TBS * tb:TBS * (tb + 1)],
                        oc_t[:])

    nc.compile()
    return nc


def _stage_inputs(hidden_states, wq, wk, wv, wo, attention_mask):
    hid = np.asarray(hidden_states, dtype=np.float32).reshape(T, D)
    # [128, (tb, dc, t_local)] : column (tb*32+dc)*512+tl = hid[tb*512+tl, dc*128+p]
    hiddenT = np.ascontiguousarray(
        hid.reshape(NTB, TBS, NDC, 128).transpose(3, 0, 2, 1).reshape(128, -1)
    ).astype(bfloat16)

    sc = 1.0 / math.sqrt(HD)
    inv_freq = 1.0 / (ROPE_THETA ** (np.arange(0, HD, 2, dtype=np.float32) / HD))
    t = np.arange(S, dtype=np.float32)
    freqs = np.outer(t, inv_freq)
    emb = np.concatenate([freqs, freqs], axis=-1)          # [S, HD]
    cosT = np.ascontiguousarray(np.cos(emb).T).astype(bfloat16)   # [HD, S]
    ssinT = np.ascontiguousarray(np.sin(emb).T)
    ssinT[:HD // 2] *= -1.0        # rotate_half sign: -sin for d<64
    ssinT = ssinT.astype(bfloat16)

    # 0/1 within-tile causal triangle: tri[k,q] = 1 iff key k <= query q
    mask = np.asarray(attention_mask, dtype=np.float32).reshape(S, S)
    tri = (mask[0:128, 0:128].T > -0.5).astype(np.float32).astype(bfloat16)
    ident = np.eye(128, dtype=np.float32).astype(bfloat16)

    wq = np.asarray(wq, dtype=np.float32)
    wk = np.asarray(wk, dtype=np.float32)
    wv = np.asarray(wv, dtype=np.float32)
    wo = np.asarray(wo, dtype=np.float32)

    in_maps = []
    for c in range(NC):
        wq_c = (wq[:, QC * c:QC * (c + 1)] * sc)
        wq_c = np.ascontiguousarray(
            wq_c.reshape(NDC, 128, QC).transpose(1, 0, 2).reshape(128, -1)
        ).astype(bfloat16)
        wk_c = np.ascontiguousarray(
            wk[:, HD * c:HD * (c + 1)].reshape(NDC, 128, HD)
            .transpose(1, 0, 2).reshape(128, -1)).astype(bfloat16)
        wv_c = np.ascontiguousarray(
            wv[:, HD * c:HD * (c + 1)].reshape(NDC, 128, HD)
            .transpose(1, 0, 2).reshape(128, -1)).astype(bfloat16)
        wo_c = np.ascontiguousarray(
            wo[QC * c:QC * (c + 1), :].reshape(HPC, 128, D)
            .transpose(1, 0, 2).reshape(128, -1)).astype(bfloat16)
        in_maps.append({
            "hiddenT": hiddenT,
            "wq": wq_c, "wk": wk_c, "wv": wv_c, "wo": wo_c,
            "cosT": cosT, "ssinT": ssinT, "tri": tri, "ident": ident,
        })
    return in_maps


def kernel(hidden_states, wq, wk, wv, wo, attention_mask, _want_trace=False):
    from concourse import bass_utils

    if "nc" not in _CACHE:
        _CACHE["nc"] = _build_nc()
    nc = _CACHE["nc"]

    in_maps = _stage_inputs(hidden_states, wq, wk, wv, wo, attention_mask)
    res = bass_utils.run_bass_kernel_spmd(
        nc, in_maps, core_ids=list(range(NC)), trace=_want_trace)
    _CACHE["last_result"] = res

    # host-side reduction of the row-sharded wo partials
    acc = np.zeros((D, T), dtype=np.float32)
    for c in range(NC):
        acc += res.results[c]["outT"].astype(np.float32)
    out = np.ascontiguousarray(acc.T).reshape(B, S, D)
    return out
